# revision 1
# baseline (speedup 1.0000x reference)
"""FFT depthwise conv == direct 7x7 circular depthwise conv, on 8 TRN2 cores.

out[b,i,j,c] = sum_{u,v} wf[c,u,v] * x[b,(i+u-3)%H,(j+v-3)%W,c],  wf = kernel[:, ::-1, ::-1]

Sharding: data-parallel over batch (1 image per core). Host pre-pads each
image circularly to (C, 230, 230) and ships it in bf16, so every on-device
tile load is a plain contiguous-row DMA (no wrap handling on device).

Per core: partitions = 64 channels x 2 row-halves; 3 channel groups x 4
row-strips of 28 rows, each processed as two 14-row sub-strips:
  TensorE : N_PE_TAPS taps as diagonal-weight bf16 matmuls, fp32-accumulated
            in PSUM per 2-row bank tile (8 rotating single-bank tiles; a
            sub-strip's 7 banks never reuse a slot mid-sub-strip, so PE
            never stalls on same-sub-strip merges)
  VectorE : remaining taps as bf16 2-op MACs (tensor_scalar product in 4x
            mode + tensor_tensor add in 2x mode), then merges each PSUM bank
            with the accumulator into a bf16 output tile (fused downcast)
  ScalarE : copies the 6 overlapping halo rows from the previous strip tile
            (saves ~18% of input DMA) and issues half the DMAs (2nd HWDGE)
Odd-column taps are 2-byte-misaligned in bf16 and would break the DVE 2x/4x
modes, so they always go to the PE side of the split. Input/output DMAs are
row-chunked across both HWDGE queues; the next group's first tile is
prefetched one chunk per strip to keep group transitions off the critical
path. Built as bacc.Bacc (finalize() spills excess per-instruction sync
waits onto EventSemaphore instructions; engine slots are tiny).
"""

import os
import sys

for _p in ("/opt/trn_rl_repo", "/root/.axon_site/_ro/trn_rl_repo"):
    if os.path.isdir(_p) and _p not in sys.path:
        sys.path.insert(0, _p)

import numpy as np

import concourse.bacc as bacc
import concourse.bass as bass
import concourse.mybir as mybir
from concourse.bass_utils import run_bass_kernel_spmd
from concourse.tile import TileContext

F32 = mybir.dt.float32
F32R = mybir.dt.float32r
BF16 = mybir.dt.bfloat16

B, H, W, C, K = 8, 224, 224, 192, 7
NCORES = 8
PAD = K // 2          # 3
PH, PW = H + 2 * PAD, W + 2 * PAD  # 230, 230 padded image dims
HALF = H // 2         # 112 output rows per s-half
TH = 28               # output rows per strip (per half)
NSTRIP = HALF // TH   # 7
CG = 64               # channels per group
NG = C // CG          # 3
TROWS = TH + 2 * PAD  # 22 input rows per strip
TCOLS = PW            # 230 input cols per strip

# --- engine tap split (tunable) -------------------------------------------
# Odd-v taps are 2-byte-misaligned in the bf16 tile, which would knock the
# DVE out of its 2x perf mode -- so they are listed first and always land on
# the PE side of the split.
ALL_TAPS = sorted(
    ((u, v) for u in range(K) for v in range(K)),
    key=lambda t: (t[1] % 2 == 0, t[0], t[1]),
)
N_PE_TAPS = 34        # taps done on TensorE via diagonal matmuls (>= 21)
SUB = 14              # sub-strip rows (= 7 PSUM banks)
PE_TAPS = ALL_TAPS[:N_PE_TAPS]
VEC_TAPS = ALL_TAPS[N_PE_TAPS:]
USE_F32R = False
USE_BF16 = True

# DMA row-chunking: each chunk is one dma_start on its own queue/engine
IN_ROW_CHUNKS = [(0, 9), (9, 9), (18, 8), (26, 8)]     # covers TROWS=34
IN_ROW_CHUNKS_TAIL = [(6, 10), (16, 9), (25, 9)]       # rows 6..34 (halo 0..6 copied on-chip)
OUT_ROW_CHUNKS = [(0, 7), (7, 7)]                      # covers SUB=14


def _tap_idx(u, v):
    return u * K + v


def _add_dep(from_inst, to_inst):
    """Ordering-only (no-semaphore) dependency between two instructions."""
    import bass_rust as _br

    fi = getattr(from_inst, "ins", from_inst)
    ti = getattr(to_inst, "ins", to_inst)
    _br.add_dep_helper(fi, ti, sync=False, reason="seed-after-merge ordering")


def build_nc():
    # Bacc (not plain Bass): its compile() runs generate_event_semaphores,
    # which spills excess per-instruction sync waits onto EventSemaphore
    # instructions -- engine instructions only have 1 inline wait slot.
    nc = bacc.Bacc()
    xdt = BF16 if USE_BF16 else (F32R if USE_F32R else F32)
    odt = BF16 if USE_BF16 else F32
    x_d = nc.declare_dram_parameter("x", [C, PH, PW], xdt, isOutput=False)
    wvec_d = nc.declare_dram_parameter("wvec", [128, NG * K * K], F32, isOutput=False)
    wdiag_d = nc.declare_dram_parameter(
        "wdiag", [128, NG, K * K, 128], xdt, isOutput=False
    )
    out_d = nc.declare_dram_parameter("out", [C, H, W], odt, isOutput=True)

    mult = mybir.AluOpType.mult
    add = mybir.AluOpType.add
    act_copy = mybir.ActivationFunctionType.Copy

    with TileContext(nc) as tc:
        with (
            tc.tile_pool(name="consts", bufs=1) as cpool,
            tc.tile_pool(name="wdg", bufs=3) as wpool,
            tc.tile_pool(name="xin", bufs=4) as xpool,
            tc.tile_pool(name="xpre", bufs=2) as prepool,
            tc.tile_pool(name="accdp", bufs=3) as adpool,
            tc.tile_pool(name="tmpp", bufs=2) as tppool,
            tc.tile_pool(name="outp", bufs=4) as opool,
            tc.tile_pool(name="psum", bufs=8, space="PSUM") as ppool,
        ):
            wvec_sb = cpool.tile([128, NG * K * K], F32)
            nc.sync.dma_start(out=wvec_sb[:], in_=wvec_d[:])

            prev_merge = [None]  # last DVE merge instruction of previous strip

            # preload ALL groups' diagonal weights up front so group
            # transitions never wait on a 1.6 MB weight DMA stuck behind
            # the queued input DMAs
            def issue_in_dma(dst_tile, g, t, chunks=IN_ROW_CHUNKS):
                xh = x_d.tensor if hasattr(x_d, "tensor") else x_d
                base = g * CG * PH * PW + t * TH * PW
                for ci, (ra, nr) in enumerate(chunks):
                    srcap = bass.AP(
                        xh,
                        base + ra * PW,
                        [[HALF * PW, 2], [PH * PW, CG], [PW, nr], [1, TCOLS]],
                    )
                    eng = nc.sync if ci % 2 == 0 else nc.scalar
                    eng.dma_start(out=dst_tile[:, ra:ra + nr, :], in_=srcap)

            # first input tile FIRST so DVE work starts immediately; weight
            # loads follow on both queues
            xt00 = xpool.tile([128, TROWS, TCOLS], xdt, name="xt0_0", tag="xt")
            issue_in_dma(xt00, 0, 0)
            wdgs = []
            for g in range(NG):
                wdg = wpool.tile([128, K * K, 128], xdt, name=f"wdg{g}", tag="wdg")
                hkk = (K * K) // 2
                nc.sync.dma_start(out=wdg[:, 0:hkk, :], in_=wdiag_d[:, g, 0:hkk, :])
                nc.scalar.dma_start(
                    out=wdg[:, hkk:, :], in_=wdiag_d[:, g, hkk:, :]
                )
                wdgs.append(wdg)

            pre_tiles = {}
            for g in range(NG):
                wdg = wdgs[g]
                # prefetch the NEXT group's first input tile, one chunk per
                # strip of this group, so the transition tile is ready early
                # without ever bursting the DMA queues
                if g + 1 < NG:
                    pre = prepool.tile(
                        [128, TROWS, TCOLS], xdt, name=f"xpre{g + 1}", tag="xpre"
                    )
                    pre_tiles[g + 1] = pre

                for t in range(NSTRIP):
                    if g + 1 < NG:
                        # one staggered prefetch chunk for (g+1, t=0)
                        ci = t
                        ra, nr = IN_ROW_CHUNKS[ci]
                        xh = x_d.tensor if hasattr(x_d, "tensor") else x_d
                        base = (g + 1) * CG * PH * PW
                        srcap = bass.AP(
                            xh,
                            base + ra * PW,
                            [[HALF * PW, 2], [PH * PW, CG], [PW, nr], [1, TCOLS]],
                        )
                        eng = nc.sync if ci % 2 == 0 else nc.scalar
                        eng.dma_start(
                            out=pre_tiles[g + 1][:, ra:ra + nr, :], in_=srcap
                        )
                    if g == 0 and t == 0:
                        xt = xt00
                    elif t == 0 and g in pre_tiles:
                        xt = pre_tiles.pop(g)
                    else:
                        xt = xpool.tile(
                            [128, TROWS, TCOLS], xdt, name=f"xt{g}_{t}", tag="xt"
                        )
                        # rows 0..5 overlap the previous strip's tail: copy
                        # them on-chip (idle ScalarE) instead of re-DMAing
                        issue_in_dma(xt, g, t, chunks=IN_ROW_CHUNKS_TAIL)
                        nc.scalar.copy(
                            out=xt[:, 0:2 * PAD, :],
                            in_=prev_xt[:, TH:TH + 2 * PAD, :],
                        )
                    prev_xt = xt

                    # ---- two 14-row sub-strips per DMA strip: each uses
                    # exactly 7 PSUM banks (the full rotation), so PE never
                    # stalls waiting for same-strip merges
                    for sub in range(TH // SUB):
                        sb = sub * SUB
                        acc = adpool.tile(
                            [128, SUB, W], BF16, name=f"acc{g}_{t}_{sub}", tag="acc"
                        )
                        outt = opool.tile(
                            [128, SUB, W], odt, name=f"outt{g}_{t}_{sub}", tag="outt"
                        )
                        tmps = [
                            tppool.tile(
                                [128, SUB, W], BF16,
                                name=f"tmp{g}_{t}_{sub}_{j}", tag=f"tmp{j}",
                            )
                            for j in range(2)
                        ]

                        # ---- vector taps on DVE: all-bf16 2-op MACs.
                        # tensor_scalar products run in 4x mode, tensor_tensor
                        # adds in 2x mode -- beats the 1x-capped fused STT.
                        u0, v0 = VEC_TAPS[0]
                        ti0 = g * K * K + _tap_idx(u0, v0)
                        wv0 = wvec_sb[:, ti0:ti0 + 1]
                        seed = nc.vector.tensor_scalar(
                            acc[:],
                            xt[:, u0 + sb:u0 + sb + SUB, v0:v0 + W],
                            wv0,
                            None,
                            mult,
                        )
                        if prev_merge[0] is not None:
                            _add_dep(seed, prev_merge[0])
                        for j, (u, v) in enumerate(VEC_TAPS[1:]):
                            ti = g * K * K + _tap_idx(u, v)
                            wv = wvec_sb[:, ti:ti + 1]
                            tmp = tmps[j % 2]
                            nc.vector.tensor_scalar(
                                tmp[:],
                                xt[:, u + sb:u + sb + SUB, v:v + W],
                                wv,
                                None,
                                mult,
                            )
                            nc.vector.tensor_tensor(acc[:], acc[:], tmp[:], add)

                        # ---- TensorE taps: SUB/2 bank-tiles of 2 rows ----
                        n_pe = len(PE_TAPS)
                        for b8 in range(SUB // 2):
                            ps = ppool.tile(
                                [128, 512], F32, name=f"ps{g}_{t}_{sub}_{b8}", tag="ps"
                            )
                            row0 = 2 * b8
                            for ti, (u, v) in enumerate(PE_TAPS):
                                rhs = xt[:, u + sb + row0:u + sb + row0 + 2, v:v + W]
                                nc.tensor.matmul(
                                    ps[:, 0:2 * W],
                                    wdg[:, _tap_idx(u, v), :],
                                    rhs,
                                    start=(ti == 0),
                                    stop=(ti == n_pe - 1),
                                )
                            # merge psum + acc -> bf16 output tile (DVE)
                            ps3 = ps[:, 0:2 * W].rearrange("p (r w) -> p r w", r=2)
                            mg = nc.vector.scalar_tensor_tensor(
                                outt[:, row0:row0 + 2, :],
                                ps3,
                                1.0,
                                acc[:, row0:row0 + 2, :],
                                mult,
                                add,
                            )
                            if b8 == 0:
                                # the ordering hint for the next seed points at
                                # the FIRST merge: enough to cover transitive
                                # PE ticks, without serializing the next
                                # sub-strip behind PE's last bank
                                prev_merge[0] = mg

                        # ---- output DMA per sub-strip, row-chunked ----
                        oh = out_d.tensor if hasattr(out_d, "tensor") else out_d
                        obase = g * CG * H * W + (t * TH + sb) * W
                        for ci, (ra, nr) in enumerate(OUT_ROW_CHUNKS):
                            dst = bass.AP(
                                oh,
                                obase + ra * W,
                                [[HALF * W, 2], [H * W, CG], [W, nr], [1, W]],
                            )
                            eng = nc.scalar if ci % 2 == 0 else nc.sync
                            eng.dma_start(out=dst, in_=outt[:, ra:ra + nr, :])
    return nc


def _host_weights(kernel):
    """kernel: (C, K, K) -> (wvec [128, NG*49], wdiag [128, NG, 49, 128])."""
    wf = kernel[:, ::-1, ::-1].astype(np.float32)  # flipped: cross-correlation form
    cl = np.arange(128) % CG  # channel-local index per partition
    wvec = np.empty((128, NG * K * K), dtype=np.float32)
    wdiag = np.zeros((128, NG, K * K, 128), dtype=np.float32)
    eye = np.arange(128)
    for g in range(NG):
        wg = wf[g * CG:(g + 1) * CG].reshape(CG, K * K)  # (64, 49)
        wvec[:, g * K * K:(g + 1) * K * K] = wg[cl]
        wdiag[eye, g, :, eye] = wg[cl]
    return wvec, wdiag


_NC_CACHE = {}


def _get_nc():
    if "nc" not in _NC_CACHE:
        nc = build_nc()
        # Bacc passes (register alloc, EventSemaphore wait-splitting, ...)
        # run in finalize(); the pjrt path serializes the module as-is, so
        # finalize here before handing it off.
        nc.finalize()
        _NC_CACHE["nc"] = nc
    return _NC_CACHE["nc"]


def run(x, kernel, trace=False, **kw):
    assert x.shape == (B, H, W, C) and kernel.shape == (C, K, K)
    nc = _get_nc()
    xT = np.ascontiguousarray(x.transpose(0, 3, 1, 2)).astype(np.float32)  # (B,C,H,W)
    xTp = np.pad(xT, ((0, 0), (0, 0), (PAD, PAD), (PAD, PAD)), mode="wrap")
    xTp = np.ascontiguousarray(xTp)
    wvec, wdiag = _host_weights(np.asarray(kernel))
    if USE_BF16:
        import ml_dtypes

        xTp = xTp.astype(ml_dtypes.bfloat16)
        wdiag = wdiag.astype(ml_dtypes.bfloat16)
    in_maps = [{"x": xTp[b], "wvec": wvec, "wdiag": wdiag} for b in range(NCORES)]
    res = run_bass_kernel_spmd(nc, in_maps, list(range(NCORES)), trace=trace, **kw)
    out = np.stack(
        [np.asarray(res.results[b]["out"]).astype(np.float32) for b in range(NCORES)]
    )
    out = np.ascontiguousarray(out.transpose(0, 2, 3, 1)).astype(np.float32)
    return out, res


def kernel(x, kernel):
    out, _ = run(np.asarray(x), np.asarray(kernel))
    return out



# revision 3
# speedup vs baseline: 2.0306x; 2.0306x over previous
"""FFT depthwise conv == direct 7x7 circular depthwise conv, on 8 TRN2 cores.

out[b,i,j,c] = sum_{u,v} wf[c,u,v] * x[b,(i+u-3)%H,(j+v-3)%W,c],  wf = kernel[:, ::-1, ::-1]

Sharding: data-parallel over batch (1 image per core). Host pre-pads each
image circularly to (C, 230, 230) bf16.

Algorithm (v2): banded-stationary matmuls with 32x32 PE array tiling.
The 7-tap column conv along H is a banded matrix contracting input rows:
for a strip of 26 output rows, out[m, x] = sum_ki A_dx[ki, m] * xin[ki, x+dx]
with A_dx[ki, m] = wf[c, ki-m, dx] (band 0..6), K=32 input rows. The 7 dx
shifts accumulate in PSUM via 7 matmuls over dx-shifted rhs windows.
tile_position=(32i, 32j) packs 16 channels concurrently on the 16 32x32
subarrays: row-group i = input partitions (4 channels per group on the free
axis), col-group j = output partitions. N=224 columns stream per matmul.

Loop: 12 rounds x 16 channels; per round 9 row-strips (8x26 + 1x16) in
blocks of [4,4,1]; per block dx=0..6 outer / strip inner so each tile's
stationary is reused across the block's strips (LDWEIGHTS amortized).
PSUM: bank(i, s%2), two strips per bank at 224-fp32 slots; accumulation
groups are per (bank, 32j partition range) - start only on the block's
first strip of each parity, overwrite-on-pending covers the second slot.
VectorE + ScalarE alternate on PSUM evacuation (fp32 -> bf16 cast).
"""

import os
import sys

for _p in ("/opt/trn_rl_repo", "/root/.axon_site/_ro/trn_rl_repo"):
    if os.path.isdir(_p) and _p not in sys.path:
        sys.path.insert(0, _p)

import numpy as np

import concourse.bacc as bacc
import concourse.bass as bass
import concourse.mybir as mybir
from concourse.bass_utils import run_bass_kernel_spmd
from concourse.tile import TileContext

F32 = mybir.dt.float32
BF16 = mybir.dt.bfloat16

B, H, W, C, K = 8, 224, 224, 192, 7
NCORES = 8
PAD = K // 2          # 3
PH = PW = H + 2 * PAD  # 230
CPR = 16              # channels per round (16 concurrent PE tiles)
ROUNDS = C // CPR     # 12
# strips: (out_row0, M out rows, K in rows)
STRIPS = [(26 * s, 26, 32) for s in range(8)] + [(208, 16, 22)]
BLOCKS = [[0, 1, 2, 3], [4, 5, 6, 7], [8]]


def _dep(later, earlier):
    """Ordering-only (no-semaphore) dependency between two instructions."""
    import bass_rust as _br

    fi = getattr(later, "ins", later)
    ti = getattr(earlier, "ins", earlier)
    _br.add_dep_helper(fi, ti, sync=False, reason="psum slot ordering")


def build_nc():
    nc = bacc.Bacc()
    x_d = nc.declare_dram_parameter("x", [C, PH, PW], BF16, isOutput=False)
    wb_d = nc.declare_dram_parameter(
        "wb", [128, ROUNDS, K, 4, 32], BF16, isOutput=False
    )
    out_d = nc.declare_dram_parameter("out", [C, H, W], BF16, isOutput=True)

    mult = mybir.AluOpType.mult

    with TileContext(nc) as tc:
        with (
            tc.tile_pool(name="consts", bufs=1) as cpool,
            tc.tile_pool(name="xin", bufs=2) as xpool,
            tc.tile_pool(name="outp", bufs=8) as opool,
            tc.tile_pool(name="psum", bufs=8, space="PSUM") as ppool,
        ):
            xh = x_d.tensor if hasattr(x_d, "tensor") else x_d
            oh = out_d.tensor if hasattr(out_d, "tensor") else out_d
            wh = wb_d.tensor if hasattr(wb_d, "tensor") else wb_d

            wbt = cpool.tile([128, ROUNDS, K, 4, 32], BF16)

            def in_dmas(xt, r, limit_s=None):
                n = 0
                for s, (row0, M, SK) in enumerate(STRIPS):
                    if limit_s is not None and s not in limit_s:
                        continue
                    for i in range(4):
                        c0 = r * CPR + 4 * i
                        base = c0 * PH * PW + row0 * PW
                        srcap = bass.AP(
                            xh, base, [[PW, SK], [PH * PW, 4], [1, PW]]
                        )
                        eng = nc.sync if n % 2 == 0 else nc.gpsimd
                        eng.dma_start(
                            out=xt[32 * i:32 * i + SK, :, s, :], in_=srcap
                        )
                        n += 1

            # round-0 input first so PE starts ASAP; weights interleaved
            nc.sync.dma_start(out=wbt[:, 0], in_=wh[:, 0])
            xt0 = xpool.tile([128, 4, 9, PW], BF16, name="xt0", tag="xt")
            in_dmas(xt0, 0)
            for r in range(1, ROUNDS):
                nc.gpsimd.dma_start(out=wbt[:, r], in_=wh[:, r])

            prev_xt = xt0
            for r in range(ROUNDS):
                if r == 0:
                    xt = xt0
                else:
                    xt = xpool.tile([128, 4, 9, PW], BF16, name=f"xt{r}", tag="xt")
                    in_dmas(xt, r)
                ne = 0
                for blk in BLOCKS:
                    parities = sorted({s % 2 for s in blk})
                    pst = {}
                    for p in parities:
                        for i in range(4):
                            pst[(i, p)] = ppool.tile(
                                [128, 512], F32,
                                name=f"ps{r}_{blk[0]}_{i}_{p}", tag="ps",
                            )
                    first_mm = {}
                    for dx in range(K):
                        for s in blk:
                            row0, M, SK = STRIPS[s]
                            p = s % 2
                            slot = (s // 2) % 2
                            sp = [t for t in blk if t % 2 == p]
                            is_first = dx == 0 and s == sp[0]
                            is_last = dx == K - 1 and s == sp[-1]
                            for i in range(4):
                                for j in range(4):
                                    mm = nc.tensor.matmul(
                                        pst[(i, p)][
                                            32 * j:32 * j + 32,
                                            slot * 224:slot * 224 + 224,
                                        ],
                                        wbt[32 * i:32 * i + SK, r, dx, j, 0:32],
                                        xt[32 * i:32 * i + SK, j, s, dx:dx + 224],
                                        start=is_first,
                                        stop=is_last,
                                        tile_position=(32 * i, 32 * j),
                                        skip_group_check=True,
                                    )
                                    if is_first:
                                        first_mm[(i, p, j)] = mm
                                    elif dx == 0 and len(sp) > 1 and s == sp[1]:
                                        # 2nd slot's first MM must not be
                                        # scheduled before the start MM that
                                        # marks the bank pending-zero
                                        _dep(mm, first_mm[(i, p, j)])

                    # ---- PSUM evacuation + output DMA ----
                    for p in parities:
                        sp = [t for t in blk if t % 2 == p]
                        width = len(sp) * 224
                        for i in range(4):
                            ot = opool.tile(
                                [128, 448], BF16,
                                name=f"ot{r}_{blk[0]}_{i}_{p}", tag="ot",
                            )
                            if ne % 2 == 0:
                                nc.vector.tensor_scalar(
                                    ot[:, 0:width],
                                    pst[(i, p)][:, 0:width],
                                    1.0, None, mult,
                                )
                            else:
                                nc.scalar.copy(
                                    out=ot[:, 0:width], in_=pst[(i, p)][:, 0:width]
                                )
                            ne += 1
                            for j in range(4):
                                c = r * CPR + 4 * i + j
                                if len(sp) == 2:
                                    r0a = STRIPS[sp[0]][0]
                                    r0b = STRIPS[sp[1]][0]
                                    Ma = STRIPS[sp[0]][1]
                                    dst = bass.AP(
                                        oh, c * H * W + r0a * W,
                                        [[W, Ma], [(r0b - r0a) * W, 2], [1, W]],
                                    )
                                    src = ot[32 * j:32 * j + Ma, :].rearrange(
                                        "p (t x) -> p t x", t=2
                                    )
                                else:
                                    r0a, Ma, _ = STRIPS[sp[0]]
                                    dst = bass.AP(
                                        oh, c * H * W + r0a * W,
                                        [[W, Ma], [1, W]],
                                    )
                                    src = ot[32 * j:32 * j + Ma, 0:224]
                                deng = nc.gpsimd if (ne + j) % 2 else nc.sync
                                deng.dma_start(out=dst, in_=src)
                prev_xt = xt
    return nc


def _host_weights(kernel):
    """kernel (C,7,7) -> wb [128, ROUNDS, 7, 4, 32] banded lhsT blocks."""
    wf = kernel[:, ::-1, ::-1].astype(np.float32)  # (C, 7, 7) flipped
    ki = np.arange(32)[:, None]
    m = np.arange(32)[None, :]
    dy = ki - m
    mask = (dy >= 0) & (dy <= 6) & (m < 26)
    dyc = np.clip(dy, 0, 6)
    wb = np.zeros((128, ROUNDS, K, 4, 32), dtype=np.float32)
    for r in range(ROUNDS):
        for i in range(4):
            for j in range(4):
                c = r * CPR + 4 * i + j
                band = wf[c]  # (7 dy, 7 dx)
                mat = np.where(mask[:, :, None], band[dyc], 0.0)  # (ki, m, dx)
                wb[32 * i:32 * i + 32, r, :, j, :] = mat.transpose(0, 2, 1)
    return wb


_NC_CACHE = {}


def _get_nc():
    if "nc" not in _NC_CACHE:
        nc = build_nc()
        nc.finalize()
        _NC_CACHE["nc"] = nc
    return _NC_CACHE["nc"]


def run(x, kernel, trace=False, **kw):
    import ml_dtypes

    assert x.shape == (B, H, W, C) and kernel.shape == (C, K, K)
    nc = _get_nc()
    xT = np.ascontiguousarray(x.transpose(0, 3, 1, 2)).astype(np.float32)
    xTp = np.pad(xT, ((0, 0), (0, 0), (PAD, PAD), (PAD, PAD)), mode="wrap")
    xTp = np.ascontiguousarray(xTp).astype(ml_dtypes.bfloat16)
    wb = _host_weights(np.asarray(kernel)).astype(ml_dtypes.bfloat16)
    in_maps = [{"x": xTp[b], "wb": wb} for b in range(NCORES)]
    res = run_bass_kernel_spmd(nc, in_maps, list(range(NCORES)), trace=trace, **kw)
    out = np.stack(
        [np.asarray(res.results[b]["out"]).astype(np.float32) for b in range(NCORES)]
    )
    out = np.ascontiguousarray(out.transpose(0, 2, 3, 1)).astype(np.float32)
    return out, res


def kernel(x, kernel):
    out, _ = run(np.asarray(x), np.asarray(kernel))
    return out


# revision 9
# speedup vs baseline: 2.3468x; 1.1557x over previous
"""FFT depthwise conv == direct 7x7 circular depthwise conv, on 8 TRN2 cores.

out[b,i,j,c] = sum_{u,v} wf[c,u,v] * x[b,(i+u-3)%H,(j+v-3)%W,c],  wf = kernel[:, ::-1, ::-1]

Sharding: data-parallel over batch (1 image per core). Host pre-pads each
image circularly to (C, 230, 230) bf16.

Algorithm (v2): banded-stationary matmuls with 32x32 PE array tiling.
The 7-tap column conv along H is a banded matrix contracting input rows:
for a strip of 26 output rows, out[m, x] = sum_ki A_dx[ki, m] * xin[ki, x+dx]
with A_dx[ki, m] = wf[c, ki-m, dx] (band 0..6), K=32 input rows. The 7 dx
shifts accumulate in PSUM via 7 matmuls over dx-shifted rhs windows.
tile_position=(32i, 32j) packs 16 channels concurrently on the 16 32x32
subarrays: row-group i = input partitions (4 channels per group on the free
axis), col-group j = output partitions. N=224 columns stream per matmul.

Loop: 12 rounds x 16 channels; per round 9 row-strips (8x26 + 1x16) in
blocks of [4,4,1]; per block dx=0..6 outer / strip inner so each tile's
stationary is reused across the block's strips (LDWEIGHTS amortized).
PSUM: bank(i, s%2), two strips per bank at 224-fp32 slots; accumulation
groups are per (bank, 32j partition range) - start only on the block's
first strip of each parity, overwrite-on-pending covers the second slot.
VectorE + ScalarE alternate on PSUM evacuation (fp32 -> bf16 cast).
"""

import os
import sys

for _p in ("/opt/trn_rl_repo", "/root/.axon_site/_ro/trn_rl_repo"):
    if os.path.isdir(_p) and _p not in sys.path:
        sys.path.insert(0, _p)

import numpy as np

import concourse.bacc as bacc
import concourse.bass as bass
import concourse.mybir as mybir
from concourse.bass_utils import run_bass_kernel_spmd
from concourse.tile import TileContext

F32 = mybir.dt.float32
BF16 = mybir.dt.bfloat16

B, H, W, C, K = 8, 224, 224, 192, 7
NCORES = 8
PAD = K // 2          # 3
PH = PW = H + 2 * PAD  # 230
CPR = 16              # channels per round (16 concurrent PE tiles)
ROUNDS = C // CPR     # 12
# strips: (out_row0, M out rows, K in rows)
STRIPS = [(26 * s, 26, 32) for s in range(8)] + [(208, 16, 22)]
BLOCKS = [[0, 1, 2, 3], [4, 5, 6, 7], [8]]


def _dep(later, earlier):
    """Ordering-only (no-semaphore) dependency between two instructions."""
    import bass_rust as _br

    fi = getattr(later, "ins", later)
    ti = getattr(earlier, "ins", earlier)
    _br.add_dep_helper(fi, ti, sync=False, reason="psum slot ordering")


# per-round output buffer layout: 20 evac slots
#   blk0 (strips 0-3): slots (i,p) i=0..3, p=0,1 -> width 448 each
#   blk1 (strips 4-7): same, offset 3584
#   blk2 (strip 8):    slots (i,0) -> width 224 each, offset 7168
OBUF_W = 16 * 448 + 4 * 224  # 8064 elements per partition


def _obuf_off(blk, i, p):
    if blk < 2:
        return blk * 3584 + (i * 2 + p) * 448
    return 7168 + i * 224


def build_nc():
    nc = bacc.Bacc()
    x_d = nc.declare_dram_parameter(
        "x", [ROUNDS, 128, 4, 9, PW], BF16, isOutput=False
    )
    wb_d = nc.declare_dram_parameter(
        "wb", [128, ROUNDS, K, 4, 32], BF16, isOutput=False
    )
    out_d = nc.declare_dram_parameter(
        "out", [ROUNDS, 128, OBUF_W], BF16, isOutput=True
    )

    mult = mybir.AluOpType.mult

    with TileContext(nc) as tc:
        with (
            tc.tile_pool(name="consts", bufs=1) as cpool,
            tc.tile_pool(name="xin", bufs=2) as xpool,
            tc.tile_pool(name="outp", bufs=2) as opool,
            tc.tile_pool(name="psum", bufs=8, space="PSUM") as ppool,
        ):
            xh = x_d.tensor if hasattr(x_d, "tensor") else x_d
            oh = out_d.tensor if hasattr(out_d, "tensor") else out_d
            wh = wb_d.tensor if hasattr(wb_d, "tensor") else wb_d

            wbt = cpool.tile([128, ROUNDS, K, 4, 32], BF16)

            # free-size per partition of one round's input tile
            XFREE = 4 * 9 * PW

            def in_dmas(xt, r):
                # 4 DMAs per round, one per 32-partition group: per-partition
                # payload is one contiguous 16.5KB run (32 descriptors each)
                for i in range(4):
                    base = (r * 128 + 32 * i) * XFREE
                    srcap = bass.AP(xh, base, [[XFREE, 32], [1, XFREE]])
                    eng = nc.sync if i % 2 == 0 else nc.gpsimd
                    eng.dma_start(out=xt[32 * i:32 * i + 32, :, :, :], in_=srcap)

            # round-0 input first so PE starts ASAP; weights interleaved
            nc.sync.dma_start(out=wbt[:, 0], in_=wh[:, 0])
            xt0 = xpool.tile([128, 4, 9, PW], BF16, name="xt0", tag="xt")
            in_dmas(xt0, 0)
            for r in range(1, ROUNDS):
                nc.gpsimd.dma_start(out=wbt[:, r], in_=wh[:, r])

            for r in range(ROUNDS):
                if r == 0:
                    xt = xt0
                else:
                    xt = xpool.tile([128, 4, 9, PW], BF16, name=f"xt{r}", tag="xt")
                    in_dmas(xt, r)
                obuf = opool.tile([128, OBUF_W], BF16, name=f"ob{r}", tag="ob")
                ne = 0
                for blk in BLOCKS:
                    parities = sorted({s % 2 for s in blk})
                    pst = {}
                    for p in parities:
                        for i in range(4):
                            pst[(i, p)] = ppool.tile(
                                [128, 512], F32,
                                name=f"ps{r}_{blk[0]}_{i}_{p}", tag="ps",
                            )
                    first_mm = {}
                    for dx in range(K):
                        for s in blk:
                            row0, M, SK = STRIPS[s]
                            p = s % 2
                            slot = (s // 2) % 2
                            sp = [t for t in blk if t % 2 == p]
                            is_first = dx == 0 and s == sp[0]
                            is_last = dx == K - 1 and s == sp[-1]
                            for i in range(4):
                                for j in range(4):
                                    mm = nc.tensor.matmul(
                                        pst[(i, p)][
                                            32 * j:32 * j + 32,
                                            slot * 224:slot * 224 + 224,
                                        ],
                                        wbt[32 * i:32 * i + SK, r, dx, j, 0:32],
                                        xt[32 * i:32 * i + SK, j, s, dx:dx + 224],
                                        start=is_first,
                                        stop=is_last,
                                        tile_position=(32 * i, 32 * j),
                                        skip_group_check=True,
                                    )
                                    if is_first:
                                        first_mm[(i, p, j)] = mm
                                    elif dx == 0 and len(sp) > 1 and s == sp[1]:
                                        # 2nd slot's first MM must not be
                                        # scheduled before the start MM that
                                        # marks the bank pending-zero
                                        _dep(mm, first_mm[(i, p, j)])

                    # ---- PSUM evacuation into the round's output buffer ----
                    bi = BLOCKS.index(blk)
                    for p in parities:
                        sp = [t for t in blk if t % 2 == p]
                        width = len(sp) * 224
                        for i in range(4):
                            off = _obuf_off(bi, i, p)
                            if ne % 2 == 0:
                                nc.vector.tensor_scalar(
                                    obuf[:, off:off + width],
                                    pst[(i, p)][:, 0:width],
                                    1.0, None, mult,
                                )
                            else:
                                nc.scalar.copy(
                                    out=obuf[:, off:off + width],
                                    in_=pst[(i, p)][:, 0:width],
                                )
                            ne += 1

                # ---- one output DMA per round half (2 queues) ----
                for half in range(2):
                    pbase = 64 * half
                    dst = bass.AP(
                        oh, (r * 128 + pbase) * OBUF_W,
                        [[OBUF_W, 64], [1, OBUF_W]],
                    )
                    eng = nc.gpsimd if half == 0 else nc.sync
                    eng.dma_start(out=dst, in_=obuf[pbase:pbase + 64, :])
    return nc


def _host_weights(kernel):
    """kernel (C,7,7) -> wb [128, ROUNDS, 7, 4, 32] banded lhsT blocks."""
    wf = kernel[:, ::-1, ::-1].astype(np.float32)  # (C, 7, 7) flipped
    ki = np.arange(32)[:, None]
    m = np.arange(32)[None, :]
    dy = ki - m
    mask = (dy >= 0) & (dy <= 6) & (m < 26)
    dyc = np.clip(dy, 0, 6)
    wb = np.zeros((128, ROUNDS, K, 4, 32), dtype=np.float32)
    for r in range(ROUNDS):
        for i in range(4):
            for j in range(4):
                c = r * CPR + 4 * i + j
                band = wf[c]  # (7 dy, 7 dx)
                mat = np.where(mask[:, :, None], band[dyc], 0.0)  # (ki, m, dx)
                wb[32 * i:32 * i + 32, r, :, j, :] = mat.transpose(0, 2, 1)
    return wb


_NC_CACHE = {}


def _get_nc():
    if "nc" not in _NC_CACHE:
        nc = build_nc()
        nc.finalize()
        _NC_CACHE["nc"] = nc
    return _NC_CACHE["nc"]


def _host_prep_x(xTp):
    """(B, C, 230, 230) padded bf16 -> (B, ROUNDS, 128, 4, 9, 230) SBUF image."""
    B_ = xTp.shape[0]
    xprep = np.zeros((B_, ROUNDS, 128, 4, 9, PW), dtype=xTp.dtype)
    for s, (row0, M, SK) in enumerate(STRIPS):
        for i in range(4):
            # channels c = r*16 + 4i + j -> view (B, ROUNDS, 4j, PH, PW)
            ch = xTp.reshape(B_, ROUNDS, 4, 4, PH, PW)[:, :, i]
            # rows row0..row0+SK to partitions 32i+ki
            xprep[:, :, 32 * i:32 * i + SK, :, s, :] = ch[
                :, :, :, row0:row0 + SK, :
            ].transpose(0, 1, 3, 2, 4)
    return xprep


def _host_decode_out(o2):
    """(B, ROUNDS, 128, OBUF_W) bf16 -> (B, C, H, W) fp32."""
    B_ = o2.shape[0]
    out = np.empty((B_, C, H, W), dtype=np.float32)
    o2f = o2.astype(np.float32)
    for bi, blk in enumerate(BLOCKS):
        parities = sorted({s % 2 for s in blk})
        for p in parities:
            sp = [t for t in blk if t % 2 == p]
            for i in range(4):
                off = _obuf_off(bi, i, p)
                for t, s in enumerate(sp):
                    row0, M, SK = STRIPS[s]
                    for j in range(4):
                        # channel c = r*16+4i+j, partitions 32j..32j+M
                        seg = o2f[
                            :, :, 32 * j:32 * j + M,
                            off + 224 * t:off + 224 * t + 224,
                        ]  # (B, ROUNDS, M, 224)
                        out[
                            :,
                            np.arange(ROUNDS) * CPR + 4 * i + j,
                            row0:row0 + M,
                            :,
                        ] = seg
    return out


def run(x, kernel, trace=False, **kw):
    import ml_dtypes

    assert x.shape == (B, H, W, C) and kernel.shape == (C, K, K)
    nc = _get_nc()
    xT = np.ascontiguousarray(x.transpose(0, 3, 1, 2)).astype(np.float32)
    xTp = np.pad(xT, ((0, 0), (0, 0), (PAD, PAD), (PAD, PAD)), mode="wrap")
    xTp = xTp.astype(ml_dtypes.bfloat16)
    xprep = _host_prep_x(xTp)
    wb = _host_weights(np.asarray(kernel)).astype(ml_dtypes.bfloat16)
    in_maps = [{"x": xprep[b], "wb": wb} for b in range(NCORES)]
    res = run_bass_kernel_spmd(nc, in_maps, list(range(NCORES)), trace=trace, **kw)
    o2 = np.stack([np.asarray(res.results[b]["out"]) for b in range(NCORES)])
    out = _host_decode_out(o2)
    out = np.ascontiguousarray(out.transpose(0, 2, 3, 1)).astype(np.float32)
    return out, res


def kernel(x, kernel):
    out, _ = run(np.asarray(x), np.asarray(kernel))
    return out


# revision 16
# speedup vs baseline: 3.6243x; 1.5443x over previous
"""FFT depthwise conv == direct 7x7 circular depthwise conv, on 8 TRN2 cores.

out[b,i,j,c] = sum_{u,v} wf[c,u,v] * x[b,(i+u-3)%H,(j+v-3)%W,c],  wf = kernel[:, ::-1, ::-1]

Sharding: data-parallel over batch (1 image per core). Host pre-pads each
image circularly and pre-arranges it into the exact SBUF layout, so every
DMA moves one contiguous multi-KB run per partition (descriptor-cheap).

Algorithm (v4): banded-stationary matmuls with 64x64 PE array tiling.
A 7-tap column conv along H is a banded matrix contracting input rows.
The 64x64 stationary is block-diagonal over TWO 26-row strips of one
channel: out[32u+m, x] = sum_ki A_dx[ki, m] xin[32u+ki, x+dx]. The 7 dx
shifts accumulate in PSUM via 7 matmuls over dx-shifted rhs windows.
tile_position=(64I, 64J) packs 4 channels concurrently on the 4 64x64
subarrays. Per channel: 5 strip-units (4 pairs + strip 8), 7 dx -> 35
matmuls; 6720 total (the per-instruction engine overhead, not FLOPs, is
the binding cost at this size).

PSUM: one bank per strip-unit wave; J selects the partition half, I the
224-column slot. One accumulation group per (bank, 64J partitions): only
the (I=0, J) dx=0 matmul uses start=True; (I=1, J) first touch relies on
pending-zero overwrite (ordering-dep enforced). VectorE + ScalarE
alternate on PSUM evacuation into a per-round output buffer shipped with
one descriptor-cheap DMA per partition-half; host inverse-permutes.
"""

import os
import sys

for _p in ("/opt/trn_rl_repo", "/root/.axon_site/_ro/trn_rl_repo"):
    if os.path.isdir(_p) and _p not in sys.path:
        sys.path.insert(0, _p)

import numpy as np

import concourse.bacc as bacc
import concourse.bass as bass
import concourse.mybir as mybir
from concourse.bass_utils import run_bass_kernel_spmd
from concourse.tile import TileContext

F32 = mybir.dt.float32
BF16 = mybir.dt.bfloat16

B, H, W, C, K = 8, 224, 224, 192, 7
NCORES = 8
PAD = K // 2          # 3
PH = PW = H + 2 * PAD  # 230
CPR = 4               # channels per round (4 concurrent 64x64 PE tiles)
ROUNDS = C // CPR     # 48
# strip-units: (row0, rows_u0, rows_u1): u-halves at partitions 32u+ki
# unit su<4: strips (2su, 2su+1): padded rows 52su+26u+ki
# unit su=4: strip 8 alone on u=0 (rows 208..229, 16 out rows), u=1 unused
NSU = 5
OBUF_W = 5 * 448  # 2240 elements per partition per round


def _obuf_off(su):
    return su * 448


def _dep(later, earlier):
    """Ordering-only (no-semaphore) dependency between two instructions."""
    import bass_rust as _br

    fi = getattr(later, "ins", later)
    ti = getattr(earlier, "ins", earlier)
    _br.add_dep_helper(fi, ti, sync=False, reason="psum slot ordering")


def build_nc():
    nc = bacc.Bacc()
    # x layout: [ROUNDS, 128, 2(J), NSU, PW] - partition p = 64I+32u+ki
    x_d = nc.declare_dram_parameter(
        "x", [ROUNDS, 128, 2, NSU, PW], BF16, isOutput=False
    )
    wb_d = nc.declare_dram_parameter(
        "wb", [128, ROUNDS, K, 2, 64], BF16, isOutput=False
    )
    out_d = nc.declare_dram_parameter(
        "out", [ROUNDS, 128, OBUF_W], BF16, isOutput=True
    )

    mult = mybir.AluOpType.mult
    XFREE = 2 * NSU * PW

    with TileContext(nc) as tc:
        with (
            tc.tile_pool(name="consts", bufs=1) as cpool,
            tc.tile_pool(name="xin", bufs=3) as xpool,
            tc.tile_pool(name="outp", bufs=3) as opool,
            tc.tile_pool(name="psum", bufs=8, space="PSUM") as ppool,
        ):
            xh = x_d.tensor if hasattr(x_d, "tensor") else x_d
            oh = out_d.tensor if hasattr(out_d, "tensor") else out_d
            wh = wb_d.tensor if hasattr(wb_d, "tensor") else wb_d

            wbt = cpool.tile([128, ROUNDS, K, 2, 64], BF16)

            def in_dmas(xt, r):
                for half in range(2):
                    base = (r * 128 + 64 * half) * XFREE
                    srcap = bass.AP(xh, base, [[XFREE, 64], [1, XFREE]])
                    eng = nc.sync if half == 0 else nc.gpsimd
                    eng.dma_start(
                        out=xt[64 * half:64 * half + 64, :, :, :], in_=srcap
                    )

            # round-0 input + weights first; weight DMAs spread over queues
            wchunk = min(6, ROUNDS)
            nc.sync.dma_start(out=wbt[:, 0:wchunk], in_=wh[:, 0:wchunk])
            xt0 = xpool.tile([128, 2, NSU, PW], BF16, name="xt0", tag="xt")
            in_dmas(xt0, 0)
            for rc in range(wchunk, ROUNDS, wchunk):
                eng = (nc.gpsimd, nc.scalar)[(rc // wchunk) % 2]
                eng.dma_start(
                    out=wbt[:, rc:rc + wchunk], in_=wh[:, rc:rc + wchunk]
                )

            for r in range(ROUNDS):
                if r == 0:
                    xt = xt0
                else:
                    xt = xpool.tile(
                        [128, 2, NSU, PW], BF16, name=f"xt{r}", tag="xt"
                    )
                    in_dmas(xt, r)
                obuf = opool.tile([128, OBUF_W], BF16, name=f"ob{r}", tag="ob")
                for su in range(NSU):
                    # one PSUM bank per row-group I: different row tiles must
                    # not access the same bank simultaneously
                    pst = [
                        ppool.tile([128, 512], F32, name=f"ps{r}_{su}_{i}",
                                   tag="ps")
                        for i in range(2)
                    ]
                    for dx in range(K):
                        for I in range(2):
                            for J in range(2):
                                nc.tensor.matmul(
                                    pst[I][64 * J:64 * J + 64, 0:224],
                                    wbt[64 * I:64 * I + 64, r, dx, J, 0:64],
                                    xt[64 * I:64 * I + 64, J, su, dx:dx + 224],
                                    start=(dx == 0),
                                    stop=(dx == K - 1),
                                    tile_position=(64 * I, 64 * J),
                                    skip_group_check=True,
                                )

                    # ---- PSUM evacuation into the round's output buffer ----
                    off = _obuf_off(su)
                    for I in range(2):
                        dst = obuf[:, off + 224 * I:off + 224 * I + 224]
                        if (su + I) % 2 == 0:
                            nc.vector.tensor_scalar(
                                dst, pst[I][:, 0:224], 1.0, None, mult,
                            )
                        else:
                            nc.scalar.copy(out=dst, in_=pst[I][:, 0:224])

                # ---- one output DMA per partition half ----
                for half in range(2):
                    pbase = 64 * half
                    dst = bass.AP(
                        oh, (r * 128 + pbase) * OBUF_W,
                        [[OBUF_W, 64], [1, OBUF_W]],
                    )
                    eng = nc.gpsimd if half == 0 else nc.sync
                    eng.dma_start(out=dst, in_=obuf[pbase:pbase + 64, :])
    return nc


def _host_weights(kernel):
    """kernel (C,7,7) -> wb [128, ROUNDS, 7, 2, 64] block-diag banded lhsT."""
    wf = kernel[:, ::-1, ::-1].astype(np.float32)  # (C, 7, 7) flipped
    ki = np.arange(32)[:, None]
    m = np.arange(32)[None, :]
    dy = ki - m
    mask = (dy >= 0) & (dy <= 6) & (m < 26)
    dyc = np.clip(dy, 0, 6)
    wb = np.zeros((128, ROUNDS, K, 2, 64), dtype=np.float32)
    for r in range(ROUNDS):
        for I in range(2):
            for J in range(2):
                c = r * CPR + 2 * I + J
                band = wf[c]  # (7 dy, 7 dx)
                mat = np.where(mask[:, :, None], band[dyc], 0.0)  # (ki,m,dx)
                matT = mat.transpose(0, 2, 1)  # (ki, dx, m)
                for u in range(2):
                    wb[64 * I + 32 * u:64 * I + 32 * u + 32, r, :, J,
                       32 * u:32 * u + 32] = matT
    return wb


def _su_rows(su, u):
    """(row0, nrows) of padded-image rows for unit su, half u; None if unused."""
    if su < 4:
        return (52 * su + 26 * u, 32)
    return (208, 22) if u == 0 else None


def _host_prep_x(xTp):
    """(B, C, 230, 230) padded bf16 -> (B, ROUNDS, 128, 2, NSU, 230)."""
    B_ = xTp.shape[0]
    ch = xTp.reshape(B_, ROUNDS, 2, 2, PH, PW)  # [b, r, I, J, row, x]
    xprep = np.zeros((B_, ROUNDS, 128, 2, NSU, PW), dtype=xTp.dtype)
    for su in range(NSU):
        for u in range(2):
            rr = _su_rows(su, u)
            if rr is None:
                continue
            row0, n = rr
            for I in range(2):
                p0 = 64 * I + 32 * u
                # (B, r, J, n, x) -> partitions p0..p0+n
                xprep[:, :, p0:p0 + n, :, su, :] = ch[
                    :, :, I, :, row0:row0 + n, :
                ].transpose(0, 1, 3, 2, 4)
    return xprep


def _host_decode_out(o2):
    """(B, ROUNDS, 128, OBUF_W) bf16 -> (B, C, H, W) fp32."""
    B_ = o2.shape[0]
    out = np.empty((B_, C, H, W), dtype=np.float32)
    o2f = o2.astype(np.float32)
    rix = np.arange(ROUNDS)
    for su in range(NSU):
        off = _obuf_off(su)
        for I in range(2):
            seg = o2f[:, :, :, off + 224 * I:off + 224 * I + 224]
            for J in range(2):
                cidx = rix * CPR + 2 * I + J
                for u in range(2):
                    if su == 4:
                        if u == 1:
                            continue
                        orow, M = 208, 16
                    else:
                        orow, M = 52 * su + 26 * u, 26
                    p0 = 64 * J + 32 * u
                    out[:, cidx, orow:orow + M, :] = seg[
                        :, :, p0:p0 + M
                    ]
    return out


_NC_CACHE = {}


def _get_nc():
    if "nc" not in _NC_CACHE:
        nc = build_nc()
        nc.finalize()
        _NC_CACHE["nc"] = nc
    return _NC_CACHE["nc"]


def run(x, kernel, trace=False, **kw):
    import ml_dtypes

    assert x.shape == (B, H, W, C) and kernel.shape == (C, K, K)
    nc = _get_nc()
    xT = np.ascontiguousarray(x.transpose(0, 3, 1, 2)).astype(np.float32)
    xTp = np.pad(xT, ((0, 0), (0, 0), (PAD, PAD), (PAD, PAD)), mode="wrap")
    xTp = xTp.astype(ml_dtypes.bfloat16)
    xprep = _host_prep_x(xTp)
    wb = _host_weights(np.asarray(kernel)).astype(ml_dtypes.bfloat16)
    in_maps = [{"x": xprep[b], "wb": wb} for b in range(NCORES)]
    res = run_bass_kernel_spmd(nc, in_maps, list(range(NCORES)), trace=trace, **kw)
    o2 = np.stack([np.asarray(res.results[b]["out"]) for b in range(NCORES)])
    out = _host_decode_out(o2)
    out = np.ascontiguousarray(out.transpose(0, 2, 3, 1)).astype(np.float32)
    return out, res


def kernel(x, kernel):
    out, _ = run(np.asarray(x), np.asarray(kernel))
    return out
